# revision 19
# baseline (speedup 1.0000x reference)
"""Group MoE layer (2 groups x 4 experts, top-1 group / top-2 expert routing)
on 8 Trainium2 NeuronCores via expert parallelism.

Strategy:
  - Host computes the (tiny) routing: language-gate argmax over groups,
    per-group expert top-k + softmax weights.
  - Tokens are dispatched by (group, expert) assignment: core c = g*4+e
    receives exactly the tokens routed to expert (g, e), padded to a common
    capacity C (SPMD: all cores run the same program).
  - Each core runs the dense FFN for its expert:
        Y^T = W2 @ relu(W1 @ X^T + b1) + b2      (tokens in the moving dim)
    with bf16 weights/activations and fp32 PSUM accumulation.
  - Capacity is padded to 4 (not 128) and split into token blocks that are
    all >=128 wide: a matmul with <128 moving columns is LDWEIGHTS-bound
    (~56ns floor vs n*0.417ns), so [384, 512, 156] beats [512, 512, 28].
  - All bulk DMA goes need-ordered on one queue (x blk0, W1 chunks small
    first, then the rest, W2 last); biases ride the scalar queue. This keeps
    the critical first ~1MB (x0 + first W1 chunk) uncontended so real
    matmuls start ~12.6us in, instead of ~17.5us when W2's 8.4MB shares
    the pipe.
  - A short burst of dummy matmuls at t=0 warms the PE HAM clock gate
    (cold PE runs at 1.2GHz for the first ~3.4us window).
  - Host scatter-adds the weighted expert outputs back into the full output.
"""

import numpy as np
import ml_dtypes

import concourse.bacc as bacc
import concourse.mybir as mybir
from concourse import tile
from concourse import bass_utils

B, L, D, H = 2, 2048, 1024, 4096
G, E = 2, 4
NCORES = G * E
PART = 128

# W1 chunk sizes in h-tiles (sum = H/128 = 32): small first so the first
# relu's weights land fast, big later to keep dma_start trigger count low.
W1_CHUNKS = [1, 1, 1, 1, 2, 2, 4, 4, 8, 8]
W2_CHUNKS = [4, 4, 4, 4, 4, 4, 4, 4]

_BF16 = ml_dtypes.bfloat16

_program_cache: dict[tuple, object] = {}


def _blocks(C: int) -> list[int]:
    """Split capacity C (multiple of 4) into moving-dim blocks, all >=128
    (below 128 columns a matmul hits the LDWEIGHTS floor) and <=448:
    n=512 blocks show a periodic ~213ns LDWEIGHTS-prefetch stall every 49
    matmuls (~2.3us over a block) that n<=448 blocks don't. A 384-token
    first block shrinks the critical startup DMA (x block 0) while still
    pacing PE weight consumption below the DMA stream rate; the small last
    block shrinks the drain tail."""
    assert C >= 128 and C % 4 == 0
    out = []
    rem = C
    if rem >= 384 + 448:
        out.append(384)
        rem -= 384
    while rem > 448 + 128:
        out.append(448)
        rem -= 448
    if rem > 448:
        out.extend([rem - 128, 128])
    else:
        out.append(rem)
    return out


def _build(C: int, d: int = D, h: int = H):
    """Build + compile the per-core expert FFN program for capacity C."""
    key = (C, d, h)
    if key in _program_cache:
        return _program_cache[key]

    nd = d // PART
    nh = h // PART
    ns = _blocks(C)
    nblk = len(ns)
    xoff = np.concatenate([[0], np.cumsum(ns)])  # token offsets per block

    # chunk maps: h-tile index -> (chunk idx, h-tile offset inside chunk)
    w1_start = np.concatenate([[0], np.cumsum(W1_CHUNKS)])
    w2_start = np.concatenate([[0], np.cumsum(W2_CHUNKS)])
    assert w1_start[-1] == nh and w2_start[-1] == nh

    bf16 = mybir.dt.bfloat16
    f32 = mybir.dt.float32

    nc = bacc.Bacc("TRN2", target_bir_lowering=False, debug=False,
                   num_devices=NCORES)

    # Merged-tile layouts: per partition row everything is contiguous, so
    # each dma_start is 128 large descriptors.
    xt = nc.dram_tensor("xt", [PART, nd * C], bf16, kind="ExternalInput")
    w1ts = [nc.dram_tensor(f"w1t{i}", [PART, nd * ch * PART], bf16,
                           kind="ExternalInput")
            for i, ch in enumerate(W1_CHUNKS)]
    w2ts = [nc.dram_tensor(f"w2t{i}", [PART, ch * d], bf16,
                           kind="ExternalInput")
            for i, ch in enumerate(W2_CHUNKS)]
    b1t = nc.dram_tensor("b1t", [PART, nh], f32, kind="ExternalInput")
    b2t = nc.dram_tensor("b2t", [PART, nd], f32, kind="ExternalInput")
    # y in bf16: halves drain DMA; final combine is on host in fp32 and the
    # 2e-2 rel tolerance dwarfs bf16 output rounding.
    yt = nc.dram_tensor("yt", [PART, nd * C], bf16, kind="ExternalOutput")

    with tile.TileContext(nc) as tc:
        with (
            tc.tile_pool(name="wpool", bufs=1) as wpool,
            tc.tile_pool(name="h1pool", bufs=nh) as h1pool,
            tc.tile_pool(name="ypool", bufs=1) as ypool,
            tc.tile_pool(name="ps1", bufs=4, space="PSUM") as ps1,
            tc.tile_pool(name="ps2", bufs=4, space="PSUM") as ps2,
        ):
            # --- PE warm-up: the HAM clock gate keeps a cold PE at 1.2GHz
            # until ~3.4us of sustained activity. Burn dummy matmuls on a
            # zeroed tile while the first DMAs are in flight so the real
            # matmuls start at 2.4GHz.
            warm = wpool.tile([PART, 256], bf16, tag="warm")
            nc.vector.memset(warm[:, :], 0.0)
            wps = ps2.tile([PART, 512], f32, tag="ps2")
            NWARM = 28  # ends ~12.9us: p75 of x0+w1c0 arrival (12.6-13.2)
            for i in range(NWARM):
                nc.tensor.matmul(wps[:, :256], warm[:, :PART], warm[:, :],
                                 start=(i == 0), stop=(i == NWARM - 1))

            x_sb = []
            for blk, n in enumerate(ns):
                t = wpool.tile([PART, nd * n], bf16, tag=f"x_{blk}")
                x_sb.append(t)
            w1_sb = []
            for i, ch in enumerate(W1_CHUNKS):
                t = wpool.tile([PART, nd * ch * PART], bf16, tag=f"w1_{i}")
                w1_sb.append(t)
            w2_sb = []
            for i, ch in enumerate(W2_CHUNKS):
                t = wpool.tile([PART, ch * d], bf16, tag=f"w2_{i}")
                w2_sb.append(t)

            # DMA plan: the sync ring alone sustains ~370-430GB/s and has the
            # lowest trigger latency; scalar/gpsimd rings ramp ~1.5us later
            # and run slower. So: x0 split sync+scalar (sync half gates the
            # first matmuls), ALL weights need-ordered on sync.
            half = (nd * ns[0]) // 2
            nc.sync.dma_start(out=x_sb[0][:, :half], in_=xt.ap()[:, :half])
            nc.scalar.dma_start(out=x_sb[0][:, half:],
                                in_=xt.ap()[:, half:nd * ns[0]])
            b1_sb = wpool.tile([PART, nh], f32, tag="b1")
            nc.scalar.dma_start(out=b1_sb[:, :], in_=b1t.ap()[:, :])
            b2_sb = wpool.tile([PART, nd], f32, tag="b2")
            nc.scalar.dma_start(out=b2_sb[:, :], in_=b2t.ap()[:, :])
            for i in range(len(W1_CHUNKS)):
                nc.sync.dma_start(out=w1_sb[i][:, :], in_=w1ts[i].ap()[:, :])
            for i in range(len(W2_CHUNKS)):
                nc.sync.dma_start(out=w2_sb[i][:, :], in_=w2ts[i].ap()[:, :])
            for blk in range(1, nblk):
                nc.sync.dma_start(
                    out=x_sb[blk][:, :],
                    in_=xt.ap()[:, nd * xoff[blk]:nd * xoff[blk + 1]])

            for blk in range(nblk):
                n = ns[blk]
                h1_tiles = []
                for hi in range(nh):
                    hc = int(np.searchsorted(w1_start, hi, "right")) - 1
                    ho = hi - w1_start[hc]
                    chw = W1_CHUNKS[hc] * PART
                    ps = ps1.tile([PART, 512], f32, tag="ps1")
                    for di in range(nd):
                        nc.tensor.matmul(
                            ps[:, :n],
                            w1_sb[hc][:, di * chw + ho * PART:
                                      di * chw + (ho + 1) * PART],
                            x_sb[blk][:, di * n:(di + 1) * n],
                            start=(di == 0), stop=(di == nd - 1),
                        )
                    h1 = h1pool.tile([PART, 512], bf16, tag="h1")
                    nc.scalar.activation(h1[:, :n], ps[:, :n],
                                         mybir.ActivationFunctionType.Relu,
                                         bias=b1_sb[:, hi:hi + 1], scale=1.0)
                    h1_tiles.append(h1)
                y = ypool.tile([PART, nd * 512], bf16, tag="y")
                for di in range(nd):
                    ps = ps2.tile([PART, 512], f32, tag="ps2")
                    for hi in range(nh):
                        gi = int(np.searchsorted(w2_start, hi, "right")) - 1
                        hj = hi - w2_start[gi]
                        nc.tensor.matmul(
                            ps[:, :n],
                            w2_sb[gi][:, hj * d + di * PART:
                                      hj * d + (di + 1) * PART],
                            h1_tiles[hi][:, :n],
                            start=(hi == 0), stop=(hi == nh - 1),
                        )
                    nc.vector.tensor_scalar_add(
                        y[:, di * n:(di + 1) * n], ps[:, :n],
                        b2_sb[:, di:di + 1])
                    # drain each d-tile as soon as it's ready (overlaps mm2,
                    # shrinks the end-of-kernel tail to one small transfer).
                    # Early blocks drain on the (idle) gpsimd ring to keep
                    # sync clear for x/W2 loads; the LAST block drains on
                    # sync, whose trigger latency is ~1us lower — that's the
                    # exec-time critical tail.
                    ring = nc.sync if blk == nblk - 1 else nc.gpsimd
                    ring.dma_start(
                        out=yt.ap()[:, nd * xoff[blk] + di * n:
                                    nd * xoff[blk] + (di + 1) * n],
                        in_=y[:, di * n:(di + 1) * n])

    nc.compile()
    _program_cache[key] = nc
    return nc


STRASSEN = True

# Strassen product -> (A-side quadrant pair, op) ; None = raw quadrant.
# Quadrant index q = 2*row_half + col_half of the STATIONARY matrix.
#   M1=(A11+A22)(B11+B22) M2=(A21+A22)B11 M3=A11(B12-B22) M4=A22(B21-B11)
#   M5=(A11+A12)B22       M6=(A21-A11)(B11+B12) M7=(A12-A22)(B21+B22)
_A_COMBO = {1: (0, 3, "add"), 2: (2, 3, "add"), 3: (0, None, None),
            4: (3, None, None), 5: (0, 1, "add"), 6: (2, 0, "sub"),
            7: (1, 3, "sub")}
# B-side (moving): (i=k-half, j=col-half) pairs; None = raw slice.
_B_COMBO = {1: ((0, 0), (1, 1), "add"), 2: ((0, 0), None, None),
            3: ((0, 1), (1, 1), "sub"), 4: ((1, 0), (0, 0), "sub"),
            5: ((1, 1), None, None), 6: ((0, 0), (0, 1), "add"),
            7: ((1, 0), (1, 1), "add")}
# fold schedule: product -> [(quadrant, op)] ; quadrants 0=C11 1=C12 2=C21
# 3=C22; op 'init' = first write (copy / bias-init), 'add'/'sub' = RMW.
_FOLDS = {1: [(0, "init"), (3, "init")], 2: [(2, "init"), (3, "sub")],
          3: [(1, "init"), (3, "add")], 4: [(0, "add"), (2, "add")],
          5: [(0, "sub"), (1, "add")], 6: [(3, "add")], 7: [(0, "add")]}


def _strassen_blocks(C):
    """Two even blocks, halves in [240, 512] to stay off the LDWEIGHTS
    floor and inside one PSUM bank."""
    assert C % 4 == 0
    b0 = (C // 2 + 2) // 4 * 4
    b1 = C - b0
    assert b0 % 2 == 0 and b1 % 2 == 0
    return [b0, b1]


def _build_strassen(C, d=D, h=H):
    """One Strassen level on both FFN matmuls: 7/8 of the direct PE work.
    W quadrants live raw in SBUF; the 5 summed stationary combos are staged
    [128,512] at a time by the vector/gpsimd engines just ahead of the PE;
    products accumulate in 8 rotating PSUM banks and are folded into bf16
    C-staging tiles incrementally (product psum freed ~0.5us after stop)."""
    key = (C, d, h, "strassen")
    if key in _program_cache:
        return _program_cache[key]

    nd = d // PART          # 8
    nh = h // PART          # 32
    kd = nd // 2            # 4  k-tiles per d-half (mm1)
    kh = nh // 2            # 16 k-tiles per h-half (mm2)
    mh = 4                  # m-tiles per group (mm1) / per d-half (mm2)
    ns = _strassen_blocks(C)
    xoff = [0, ns[0]]
    N2MAX = (max(ns) // 2 + 7) // 8 * 8

    bf16 = mybir.dt.bfloat16
    f32 = mybir.dt.float32

    nc = bacc.Bacc("TRN2", target_bir_lowering=False, debug=False,
                   num_devices=NCORES)

    xt = nc.dram_tensor("xt", [PART, nd * C], bf16, kind="ExternalInput")
    # W1 quadrants: [part, (mg,kt,512)] ; W2 quadrants: [part, (kt,512)]
    w1qt = [nc.dram_tensor(f"w1q{q}", [PART, 4 * kd * 512], bf16,
                           kind="ExternalInput") for q in range(4)]
    w2qt = [nc.dram_tensor(f"w2q{q}", [PART, kh * 512], bf16,
                           kind="ExternalInput") for q in range(4)]
    b1t = nc.dram_tensor("b1t", [PART, nh], f32, kind="ExternalInput")
    b2t = nc.dram_tensor("b2t", [PART, nd], f32, kind="ExternalInput")
    yt = nc.dram_tensor("yt", [PART, nd * C], bf16, kind="ExternalOutput")

    with tile.TileContext(nc) as tc:
        with (
            tc.tile_pool(name="wpool", bufs=1) as wpool,
            tc.tile_pool(name="xpool", bufs=1) as xpool,
            tc.tile_pool(name="bxpool", bufs=5) as bxpool,
            tc.tile_pool(name="stpool", bufs=6) as stpool,
            tc.tile_pool(name="bhpool", bufs=6) as bhpool,
            tc.tile_pool(name="cpool", bufs=8) as cpool,
            tc.tile_pool(name="h1pool", bufs=nh) as h1pool,
            tc.tile_pool(name="pspool", bufs=8, space="PSUM") as pspool,
        ):
            warm = wpool.tile([PART, 256], bf16, tag="warm")
            nc.vector.memset(warm[:, :], 0.0)
            wps = pspool.tile([PART, 512], f32, tag="ps")
            NWARM = 28
            for i in range(NWARM):
                nc.tensor.matmul(wps[:, :256], warm[:, :PART], warm[:, :],
                                 start=(i == 0), stop=(i == NWARM - 1))

            # resident W quadrants
            w1q = [wpool.tile([PART, kd, 4 * 512], bf16, tag=f"w1q{q}")
                   for q in range(4)]
            w2q = [wpool.tile([PART, kh, 512], bf16, tag=f"w2q{q}")
                   for q in range(4)]
            b1_sb = wpool.tile([PART, nh], f32, tag="b1")
            b2_sb = wpool.tile([PART, nd], f32, tag="b2")

            # per-block x tiles allocated on first use (pool bufs=1)
            x_sb = {}
            for b, n in enumerate(ns):
                x_sb[b] = xpool.tile([PART, nd, n], bf16, tag="x")

            # ---- DMA (sync = fast ring carries all weights need-ordered;
            # x block 0 halves on sync+scalar like the direct kernel) ----
            n0 = ns[0]
            nc.sync.dma_start(out=x_sb[0][:, 0:kd, :],
                              in_=xt.ap()[:, :kd * n0])
            nc.scalar.dma_start(out=x_sb[0][:, kd:nd, :],
                                in_=xt.ap()[:, kd * n0:nd * n0])
            nc.scalar.dma_start(out=b1_sb[:, :], in_=b1t.ap()[:, :])
            nc.scalar.dma_start(out=b2_sb[:, :], in_=b2t.ap()[:, :])
            # W1: mg0 in fine (kt, q) pieces feeding the first staging ops,
            # then full-mg per-quadrant chunks. Order q0,q3,q2,q1 (= first
            # products' needs).
            QORD = [0, 3, 2, 1]
            for kt in range(kd):
                for q in QORD:
                    nc.sync.dma_start(
                        out=w1q[q][:, kt:kt + 1, 0:512],
                        in_=w1qt[q].ap()[:, kt * 512:(kt + 1) * 512])
            for mg in range(1, 4):
                for q in QORD:
                    nc.sync.dma_start(
                        out=w1q[q][:, 0:kd, mg * 512:(mg + 1) * 512],
                        in_=w1qt[q].ap()[:, mg * kd * 512:
                                         (mg + 1) * kd * 512])
            for q in QORD:
                nc.sync.dma_start(out=w2q[q][:, :, :], in_=w2qt[q].ap()[:, :])
            nc.sync.dma_start(out=x_sb[1][:, :, :],
                              in_=xt.ap()[:, nd * ns[0]:nd * C])

            for b, n in enumerate(ns):
                n2 = n // 2
                xb = x_sb[b]

                # ---- x-side combos (per k-tile so they unlock with the
                # arriving x stream) ----
                bx = {}
                for p, (sa, sb_, op) in _B_COMBO.items():
                    if sb_ is None:
                        continue
                    t = bxpool.tile([PART, kd, N2MAX], bf16, tag=f"bx{p}")
                    bx[p] = t
                for kt in range(kd):
                    eng = nc.vector if kt % 2 == 0 else nc.gpsimd
                    for p, (sa, sb_, op) in _B_COMBO.items():
                        if sb_ is None:
                            continue
                        (ia, ja), (ib, jb) = sa, sb_
                        alu = (mybir.AluOpType.add if op == "add"
                               else mybir.AluOpType.subtract)
                        eng.tensor_tensor(
                            bx[p][:, kt:kt + 1, 0:n2],
                            xb[:, ia * kd + kt:ia * kd + kt + 1,
                               ja * n2:(ja + 1) * n2],
                            xb[:, ib * kd + kt:ib * kd + kt + 1,
                               jb * n2:(jb + 1) * n2],
                            alu)

                def moving1(p, kt):
                    sa, sb_, _ = _B_COMBO[p]
                    if sb_ is None:
                        i, j = sa
                        return xb[:, i * kd + kt:i * kd + kt + 1,
                                  j * n2:(j + 1) * n2]
                    return bx[p][:, kt:kt + 1, 0:n2]

                # ================= mm1 (Strassen) =================
                h1_tiles = [h1pool.tile([PART, n], bf16, tag="h1")
                            for _ in range(nh)]
                for mg in range(4):
                    cst = [cpool.tile([PART, mh, N2MAX], bf16, tag=f"c{qd}")
                           for qd in range(4)]
                    for p in range(1, 8):
                        qa, qb, op = _A_COMBO[p]
                        peng = nc.vector if p in (1, 5, 7) else nc.gpsimd
                        if qb is None:
                            stat = w1q[qa]
                            scol = mg * 512
                            st = None
                        else:
                            st = stpool.tile([PART, kd, 512], bf16,
                                             tag="st")
                            alu = (mybir.AluOpType.add if op == "add"
                                   else mybir.AluOpType.subtract)
                            for kt in range(kd):
                                peng.tensor_tensor(
                                    st[:, kt:kt + 1, :],
                                    w1q[qa][:, kt:kt + 1,
                                            mg * 512:(mg + 1) * 512],
                                    w1q[qb][:, kt:kt + 1,
                                            mg * 512:(mg + 1) * 512],
                                    alu)
                        ps = [pspool.tile([PART, 512], f32, tag="ps")
                              for _ in range(mh)]
                        for kt in range(kd):
                            mv = moving1(p, kt)
                            for m in range(mh):
                                if st is not None:
                                    lhsT = st[:, kt:kt + 1,
                                              m * PART:(m + 1) * PART]
                                else:
                                    lhsT = stat[:, kt:kt + 1,
                                                scol + m * PART:
                                                scol + (m + 1) * PART]
                                nc.tensor.matmul(
                                    ps[m][:, :n2], lhsT, mv,
                                    start=(kt == 0), stop=(kt == kd - 1))
                        # fold products into C staging (C11/C12 on vector,
                        # C21/C22 on gpsimd)
                        for m in range(mh):
                            for qd, op2 in _FOLDS[p]:
                                feng = nc.vector if qd < 2 else nc.gpsimd
                                dst = cst[qd][:, m:m + 1, 0:n2]
                                if op2 == "init":
                                    feng.tensor_copy(dst, ps[m][:, :n2])
                                else:
                                    alu2 = (mybir.AluOpType.add
                                            if op2 == "add"
                                            else mybir.AluOpType.subtract)
                                    feng.tensor_tensor(dst, dst,
                                                       ps[m][:, :n2], alu2)
                    # relu + bias into h1
                    for m in range(mh):
                        for i in range(2):        # h half
                            hi = i * kh // 4 * 4 + mg * mh + m
                            hi = i * (nh // 2) + mg * mh + m
                            for j in range(2):    # token half
                                qd = i * 2 + j
                                nc.scalar.activation(
                                    h1_tiles[hi][:, j * n2:(j + 1) * n2],
                                    cst[qd][:, m:m + 1, 0:n2],
                                    mybir.ActivationFunctionType.Relu,
                                    bias=b1_sb[:, hi:hi + 1], scale=1.0)

                # ================= mm2 (Strassen) =================
                def movingh(i, j, kt):
                    return h1_tiles[i * kh + kt][:, j * n2:(j + 1) * n2]

                yst = [cpool.tile([PART, mh, N2MAX], bf16, tag=f"y{qd}")
                       for qd in range(4)]
                for p in range(1, 8):
                    qa, qb, op = _A_COMBO[p]
                    peng = nc.vector if p in (1, 5, 7) else nc.gpsimd
                    ps = [pspool.tile([PART, 512], f32, tag="ps")
                          for _ in range(mh)]
                    for kt in range(kh):
                        # stationary [128, 512] for this (p, kt)
                        if qb is None:
                            st = None
                        else:
                            st = st2pool.tile([PART, 1, 512], bf16,
                                              tag="st2")
                            alu = (mybir.AluOpType.add if op == "add"
                                   else mybir.AluOpType.subtract)
                            peng.tensor_tensor(
                                st[:, :, :], w2q[qa][:, kt:kt + 1, :],
                                w2q[qb][:, kt:kt + 1, :], alu)
                        # moving [128, n2] for this (p, kt)
                        sa, sb_, bop = _B_COMBO[p]
                        if sb_ is None:
                            mv = movingh(sa[0], sa[1], kt)
                        else:
                            bh = bhpool.tile([PART, N2MAX], bf16, tag="bh")
                            alu = (mybir.AluOpType.add if bop == "add"
                                   else mybir.AluOpType.subtract)
                            beng = nc.vector if kt % 2 == 0 else nc.gpsimd
                            beng.tensor_tensor(
                                bh[:, 0:n2],
                                movingh(sa[0], sa[1], kt),
                                movingh(sb_[0], sb_[1], kt), alu)
                            mv = bh[:, 0:n2]
                        for m in range(mh):
                            if st is None:
                                lhsT = w2q[qa][:, kt:kt + 1,
                                               m * PART:(m + 1) * PART]
                            else:
                                lhsT = st[:, 0:1, m * PART:(m + 1) * PART]
                            nc.tensor.matmul(
                                ps[m][:, :n2], lhsT, mv,
                                start=(kt == 0), stop=(kt == kh - 1))
                    for m in range(mh):
                        for qd, op2 in _FOLDS[p]:
                            feng = nc.vector if qd < 2 else nc.gpsimd
                            dst = yst[qd][:, m:m + 1, 0:n2]
                            if op2 == "init":
                                # fold the output bias into the first write
                                di = (qd // 2) * mh + m
                                feng.tensor_scalar_add(
                                    dst, ps[m][:, :n2], b2_sb[:, di:di + 1])
                            else:
                                alu2 = (mybir.AluOpType.add if op2 == "add"
                                        else mybir.AluOpType.subtract)
                                feng.tensor_tensor(dst, dst, ps[m][:, :n2],
                                                   alu2)
                # drain y quadrants (last block on the low-latency sync ring)
                for qd in range(4):
                    i, j = qd // 2, qd % 2
                    for m in range(mh):
                        di = i * mh + m
                        ring = nc.sync if b == len(ns) - 1 else nc.gpsimd
                        ring.dma_start(
                            out=yt.ap()[:, nd * xoff[b] + di * n + j * n2:
                                        nd * xoff[b] + di * n + (j + 1) * n2],
                            in_=yst[qd][:, m:m + 1, 0:n2])

    nc.compile()
    _program_cache[key] = nc
    return nc


def _route(x, bn, Wlg, blg, Wg, k):
    """Numpy replica of the reference routing. Returns per-(g,e) assignment."""
    glog = bn @ Wlg.T + blg                       # (N, G)
    sel_group = np.argmax(glog, axis=1)           # (N,)
    assign = []
    for g in range(Wg.shape[0]):
        logits = x @ Wg[g].T                      # (N, E)
        order = np.argsort(-logits, axis=1, kind="stable")
        sel = order[:, :k]                        # (N, k)
        top = np.take_along_axis(logits, sel, axis=1).astype(np.float32)
        m = top.max(axis=1, keepdims=True)
        ex = np.exp(top - m)
        w = ex / ex.sum(axis=1, keepdims=True)    # (N, k)
        assign.append((sel, w))
    return sel_group, assign


def _pack_x(X, d, ns):
    """(C, d) fp32 -> [128, nd*C] bf16 merged-tile layout, block-major:
    per partition row: [blk][di][token]."""
    nd = d // PART
    xt = X.T.astype(_BF16)                        # (d, C)
    parts = []
    c0 = 0
    for n in ns:
        blk = xt[:, c0:c0 + n].reshape(nd, PART, n).transpose(1, 0, 2)
        parts.append(blk.reshape(PART, nd * n))
        c0 += n
    return np.ascontiguousarray(np.concatenate(parts, axis=1))


def _pack_w1(W1e, d):
    """(h, d) -> per-chunk [128, nd*ch*128] bf16: per partition row
    [di][h cols of chunk]."""
    nd = d // PART
    w = W1e.T.astype(_BF16)                       # (d, h)
    outs = []
    h0 = 0
    for ch in W1_CHUNKS:
        cw = ch * PART
        c = w[:, h0:h0 + cw].reshape(nd, PART, cw).transpose(1, 0, 2)
        outs.append(np.ascontiguousarray(c.reshape(PART, nd * cw)))
        h0 += cw
    return outs


def _pack_w2(W2e, d):
    """(d, h) -> per-chunk [128, ch*d] bf16: per partition row
    [hj][d cols]."""
    w = W2e.T.astype(_BF16)                       # (h, d)
    outs = []
    h0 = 0
    for ch in W2_CHUNKS:
        c = w[h0 * PART:(h0 + ch) * PART, :].reshape(ch, PART, d)
        outs.append(np.ascontiguousarray(
            c.transpose(1, 0, 2).reshape(PART, ch * d)))
        h0 += ch
    return outs


def _pack_w1q(W1e):
    """(H, D) -> 4 quadrant arrays [128, 4mg*4kt*512] bf16 for Strassen.
    dram[q][part, (mg*4+kt)*512+u] = W1[qh*2048+mg*512+u, qd*512+kt*128+part]
    """
    outs = []
    for q in range(4):
        qh, qd = q // 2, q % 2
        sub = W1e[qh * 2048:(qh + 1) * 2048, qd * 512:(qd + 1) * 512]
        arr = sub.reshape(4, 512, 4, PART).transpose(3, 0, 2, 1)
        outs.append(np.ascontiguousarray(
            arr.reshape(PART, 4 * 4 * 512).astype(_BF16)))
    return outs


def _pack_w2q(W2e):
    """(D, H) -> 4 quadrant arrays [128, 16kt*512] bf16 for Strassen.
    dram[q][part, kt*512+u] = W2[qd*512+u, qh*2048+kt*128+part], q=2*qd+qh.
    """
    outs = []
    for q in range(4):
        qd, qh = q // 2, q % 2
        sub = W2e[qd * 512:(qd + 1) * 512, qh * 2048:(qh + 1) * 2048]
        arr = sub.reshape(512, 16, PART).transpose(2, 1, 0)
        outs.append(np.ascontiguousarray(
            arr.reshape(PART, 16 * 512).astype(_BF16)))
    return outs


def _unpack_y(yt, d, ns):
    """[128, nd*C] bf16 -> (d, C) f32."""
    nd = d // PART
    yt = np.asarray(yt)
    if yt.dtype != np.float32:
        yt = yt.astype(np.float32)
    out = np.empty((d, int(sum(ns))), np.float32)
    c0 = 0
    o0 = 0
    for n in ns:
        blk = yt[:, o0:o0 + nd * n].reshape(PART, nd, n).transpose(1, 0, 2)
        out[:, c0:c0 + n] = blk.reshape(d, n)
        c0 += n
        o0 += nd * n
    return out


def kernel(**inputs) -> np.ndarray:
    xs = np.asarray(inputs["xs"], np.float32)
    bn = np.asarray(inputs["bottle_neck"], np.float32)
    Wlg = np.asarray(inputs["Wlg"], np.float32)
    blg = np.asarray(inputs["blg"], np.float32)
    Wg = np.asarray(inputs["Wg"], np.float32)
    W1 = np.asarray(inputs["W1"], np.float32)
    b1 = np.asarray(inputs["b1"], np.float32)
    W2 = np.asarray(inputs["W2"], np.float32)
    b2 = np.asarray(inputs["b2"], np.float32)
    k = int(np.asarray(inputs["top_k"]))

    Bx, Lx, d = xs.shape
    hdim = W1.shape[2]
    N = Bx * Lx
    nh = hdim // PART
    nd = d // PART
    x = xs.reshape(N, d)
    bnf = bn.reshape(N, d)

    sel_group, assign = _route(x, bnf, Wlg, blg, Wg, k)

    # Token sets per (group, expert) core.
    idxs, wgts = [], []
    for c in range(NCORES):
        g, e = divmod(c, E)
        sel, w = assign[g]
        mask = (sel_group == g)[:, None] & (sel == e)
        rows, cols = np.nonzero(mask)
        idxs.append(rows)
        wgts.append(w[rows, cols])

    cnt_max = max(len(i) for i in idxs)
    C = max(PART, -(-cnt_max // 4) * 4)           # pad capacity to 4
    use_strassen = (STRASSEN and d == D and hdim == H
                    and 968 <= C <= 2040)
    if use_strassen:
        ns = _strassen_blocks(C)
        nc = _build_strassen(C, d, hdim)
    else:
        ns = _blocks(C)
        nc = _build(C, d, hdim)

    in_maps = []
    for c in range(NCORES):
        g, e = divmod(c, E)
        cnt = len(idxs[c])
        X = np.zeros((C, d), np.float32)
        if cnt:
            X[:cnt] = x[idxs[c]]
        m = {
            "xt": _pack_x(X, d, ns),
            "b1t": np.ascontiguousarray(b1[g, e].reshape(nh, PART).T),
            "b2t": np.ascontiguousarray(b2[g, e].reshape(nd, PART).T),
        }
        if use_strassen:
            for q, arr in enumerate(_pack_w1q(W1[g, e])):
                m[f"w1q{q}"] = arr
            for q, arr in enumerate(_pack_w2q(W2[g, e])):
                m[f"w2q{q}"] = arr
        else:
            for i, arr in enumerate(_pack_w1(W1[g, e], d)):
                m[f"w1t{i}"] = arr
            for i, arr in enumerate(_pack_w2(W2[g, e], d)):
                m[f"w2t{i}"] = arr
        in_maps.append(m)

    res = bass_utils.run_bass_kernel_spmd(nc, in_maps, core_ids=list(range(NCORES)))

    out = np.zeros((N, d), np.float32)
    for c in range(NCORES):
        cnt = len(idxs[c])
        if cnt == 0:
            continue
        yc = _unpack_y(res.results[c]["yt"], d, ns)[:, :cnt].T
        out[idxs[c]] += wgts[c][:, None] * yc
    return out.reshape(Bx, Lx, d).astype(np.float32)



# revision 36
# speedup vs baseline: 1.2176x; 1.2176x over previous
"""Group MoE layer (2 groups x 4 experts, top-1 group / top-2 expert routing)
on 8 Trainium2 NeuronCores via expert parallelism.

Strategy:
  - Host computes the (tiny) routing: language-gate argmax over groups,
    per-group expert top-k + softmax weights.
  - Tokens are dispatched by (group, expert) assignment: core c = g*4+e
    receives exactly the tokens routed to expert (g, e), padded to a common
    capacity C (SPMD: all cores run the same program).
  - Each core runs the dense FFN for its expert:
        Y^T = W2 @ relu(W1 @ X^T + b1) + b2      (tokens in the moving dim)
    with bf16 weights/activations and fp32 PSUM accumulation.
  - Capacity is padded to 4 (not 128) and split into token blocks that are
    all >=128 wide: a matmul with <128 moving columns is LDWEIGHTS-bound
    (~56ns floor vs n*0.417ns), so [384, 512, 156] beats [512, 512, 28].
  - All bulk DMA goes need-ordered on one queue (x blk0, W1 chunks small
    first, then the rest, W2 last); biases ride the scalar queue. This keeps
    the critical first ~1MB (x0 + first W1 chunk) uncontended so real
    matmuls start ~12.6us in, instead of ~17.5us when W2's 8.4MB shares
    the pipe.
  - A short burst of dummy matmuls at t=0 warms the PE HAM clock gate
    (cold PE runs at 1.2GHz for the first ~3.4us window).
  - Host scatter-adds the weighted expert outputs back into the full output.
"""

import numpy as np
import ml_dtypes

import concourse.bacc as bacc
import concourse.mybir as mybir
from concourse import tile
from concourse import bass_utils

B, L, D, H = 2, 2048, 1024, 4096
G, E = 2, 4
NCORES = G * E
PART = 128

# W1 chunk sizes in h-tiles (sum = H/128 = 32): small first so the first
# relu's weights land fast, big later to keep dma_start trigger count low.
W1_CHUNKS = [1, 1, 1, 1, 2, 2, 4, 4, 8, 8]
W2_CHUNKS = [4, 4, 4, 4, 4, 4, 4, 4]

_BF16 = ml_dtypes.bfloat16

_program_cache: dict[tuple, object] = {}


def _blocks(C: int) -> list[int]:
    """Split capacity C (multiple of 4) into moving-dim blocks, all >=128
    (below 128 columns a matmul hits the LDWEIGHTS floor) and <=448:
    n=512 blocks show a periodic ~213ns LDWEIGHTS-prefetch stall every 49
    matmuls (~2.3us over a block) that n<=448 blocks don't. A 384-token
    first block shrinks the critical startup DMA (x block 0) while still
    pacing PE weight consumption below the DMA stream rate; the small last
    block shrinks the drain tail."""
    assert C >= 128 and C % 4 == 0
    out = []
    rem = C
    if rem >= 384 + 448:
        out.append(384)
        rem -= 384
    while rem > 448 + 128:
        out.append(448)
        rem -= 448
    if rem > 448:
        out.extend([rem - 128, 128])
    else:
        out.append(rem)
    return out


def _build(C: int, d: int = D, h: int = H):
    """Build + compile the per-core expert FFN program for capacity C."""
    key = (C, d, h)
    if key in _program_cache:
        return _program_cache[key]

    nd = d // PART
    nh = h // PART
    ns = _blocks(C)
    nblk = len(ns)
    xoff = np.concatenate([[0], np.cumsum(ns)])  # token offsets per block

    # chunk maps: h-tile index -> (chunk idx, h-tile offset inside chunk)
    w1_start = np.concatenate([[0], np.cumsum(W1_CHUNKS)])
    w2_start = np.concatenate([[0], np.cumsum(W2_CHUNKS)])
    assert w1_start[-1] == nh and w2_start[-1] == nh

    bf16 = mybir.dt.bfloat16
    f32 = mybir.dt.float32

    nc = bacc.Bacc("TRN2", target_bir_lowering=False, debug=False,
                   num_devices=NCORES)

    # Merged-tile layouts: per partition row everything is contiguous, so
    # each dma_start is 128 large descriptors.
    xt = nc.dram_tensor("xt", [PART, nd * C], bf16, kind="ExternalInput")
    w1ts = [nc.dram_tensor(f"w1t{i}", [PART, nd * ch * PART], bf16,
                           kind="ExternalInput")
            for i, ch in enumerate(W1_CHUNKS)]
    w2ts = [nc.dram_tensor(f"w2t{i}", [PART, ch * d], bf16,
                           kind="ExternalInput")
            for i, ch in enumerate(W2_CHUNKS)]
    b1t = nc.dram_tensor("b1t", [PART, nh], f32, kind="ExternalInput")
    b2t = nc.dram_tensor("b2t", [PART, nd], f32, kind="ExternalInput")
    # y in bf16: halves drain DMA; final combine is on host in fp32 and the
    # 2e-2 rel tolerance dwarfs bf16 output rounding.
    yt = nc.dram_tensor("yt", [PART, nd * C], bf16, kind="ExternalOutput")

    with tile.TileContext(nc) as tc:
        with (
            tc.tile_pool(name="wpool", bufs=1) as wpool,
            tc.tile_pool(name="h1pool", bufs=nh) as h1pool,
            tc.tile_pool(name="ypool", bufs=1) as ypool,
            tc.tile_pool(name="ps1", bufs=4, space="PSUM") as ps1,
            tc.tile_pool(name="ps2", bufs=4, space="PSUM") as ps2,
        ):
            # --- PE warm-up: the HAM clock gate keeps a cold PE at 1.2GHz
            # until ~3.4us of sustained activity. Burn dummy matmuls on a
            # zeroed tile while the first DMAs are in flight so the real
            # matmuls start at 2.4GHz.
            warm = wpool.tile([PART, 256], bf16, tag="warm")
            nc.vector.memset(warm[:, :], 0.0)
            wps = ps2.tile([PART, 512], f32, tag="ps2")
            NWARM = 28  # ends ~12.9us: p75 of x0+w1c0 arrival (12.6-13.2)
            for i in range(NWARM):
                nc.tensor.matmul(wps[:, :256], warm[:, :PART], warm[:, :],
                                 start=(i == 0), stop=(i == NWARM - 1))

            x_sb = []
            for blk, n in enumerate(ns):
                t = wpool.tile([PART, nd * n], bf16, tag=f"x_{blk}")
                x_sb.append(t)
            w1_sb = []
            for i, ch in enumerate(W1_CHUNKS):
                t = wpool.tile([PART, nd * ch * PART], bf16, tag=f"w1_{i}")
                w1_sb.append(t)
            w2_sb = []
            for i, ch in enumerate(W2_CHUNKS):
                t = wpool.tile([PART, ch * d], bf16, tag=f"w2_{i}")
                w2_sb.append(t)

            # DMA plan: the sync ring alone sustains ~370-430GB/s and has the
            # lowest trigger latency; scalar/gpsimd rings ramp ~1.5us later
            # and run slower. So: x0 split sync+scalar (sync half gates the
            # first matmuls), ALL weights need-ordered on sync.
            half = (nd * ns[0]) // 2
            nc.sync.dma_start(out=x_sb[0][:, :half], in_=xt.ap()[:, :half])
            nc.scalar.dma_start(out=x_sb[0][:, half:],
                                in_=xt.ap()[:, half:nd * ns[0]])
            b1_sb = wpool.tile([PART, nh], f32, tag="b1")
            nc.scalar.dma_start(out=b1_sb[:, :], in_=b1t.ap()[:, :])
            b2_sb = wpool.tile([PART, nd], f32, tag="b2")
            nc.scalar.dma_start(out=b2_sb[:, :], in_=b2t.ap()[:, :])
            for i in range(len(W1_CHUNKS)):
                nc.sync.dma_start(out=w1_sb[i][:, :], in_=w1ts[i].ap()[:, :])
            for i in range(len(W2_CHUNKS)):
                nc.sync.dma_start(out=w2_sb[i][:, :], in_=w2ts[i].ap()[:, :])
            for blk in range(1, nblk):
                nc.sync.dma_start(
                    out=x_sb[blk][:, :],
                    in_=xt.ap()[:, nd * xoff[blk]:nd * xoff[blk + 1]])

            for blk in range(nblk):
                n = ns[blk]
                h1_tiles = []
                for hi in range(nh):
                    hc = int(np.searchsorted(w1_start, hi, "right")) - 1
                    ho = hi - w1_start[hc]
                    chw = W1_CHUNKS[hc] * PART
                    ps = ps1.tile([PART, 512], f32, tag="ps1")
                    for di in range(nd):
                        nc.tensor.matmul(
                            ps[:, :n],
                            w1_sb[hc][:, di * chw + ho * PART:
                                      di * chw + (ho + 1) * PART],
                            x_sb[blk][:, di * n:(di + 1) * n],
                            start=(di == 0), stop=(di == nd - 1),
                        )
                    h1 = h1pool.tile([PART, 512], bf16, tag="h1")
                    nc.scalar.activation(h1[:, :n], ps[:, :n],
                                         mybir.ActivationFunctionType.Relu,
                                         bias=b1_sb[:, hi:hi + 1], scale=1.0)
                    h1_tiles.append(h1)
                y = ypool.tile([PART, nd * 512], bf16, tag="y")
                for di in range(nd):
                    ps = ps2.tile([PART, 512], f32, tag="ps2")
                    for hi in range(nh):
                        gi = int(np.searchsorted(w2_start, hi, "right")) - 1
                        hj = hi - w2_start[gi]
                        nc.tensor.matmul(
                            ps[:, :n],
                            w2_sb[gi][:, hj * d + di * PART:
                                      hj * d + (di + 1) * PART],
                            h1_tiles[hi][:, :n],
                            start=(hi == 0), stop=(hi == nh - 1),
                        )
                    nc.vector.tensor_scalar_add(
                        y[:, di * n:(di + 1) * n], ps[:, :n],
                        b2_sb[:, di:di + 1])
                    # drain each d-tile as soon as it's ready (overlaps mm2,
                    # shrinks the end-of-kernel tail to one small transfer).
                    # Early blocks drain on the (idle) gpsimd ring to keep
                    # sync clear for x/W2 loads; the LAST block drains on
                    # sync, whose trigger latency is ~1us lower — that's the
                    # exec-time critical tail.
                    ring = nc.sync if blk == nblk - 1 else nc.gpsimd
                    ring.dma_start(
                        out=yt.ap()[:, nd * xoff[blk] + di * n:
                                    nd * xoff[blk] + (di + 1) * n],
                        in_=y[:, di * n:(di + 1) * n])

    nc.compile()
    _program_cache[key] = nc
    return nc


STRASSEN = True

# Strassen product -> (A-side quadrant pair, op) ; None = raw quadrant.
# Quadrant index q = 2*row_half + col_half of the STATIONARY matrix.
#   M1=(A11+A22)(B11+B22) M2=(A21+A22)B11 M3=A11(B12-B22) M4=A22(B21-B11)
#   M5=(A11+A12)B22       M6=(A21-A11)(B11+B12) M7=(A12-A22)(B21+B22)
_A_COMBO = {1: (0, 3, "add"), 2: (2, 3, "add"), 3: (0, None, None),
            4: (3, None, None), 5: (0, 1, "add"), 6: (2, 0, "sub"),
            7: (1, 3, "sub")}
# B-side (moving): (i=k-half, j=col-half) pairs; None = raw slice.
_B_COMBO = {1: ((0, 0), (1, 1), "add"), 2: ((0, 0), None, None),
            3: ((0, 1), (1, 1), "sub"), 4: ((1, 0), (0, 0), "sub"),
            5: ((1, 1), None, None), 6: ((0, 0), (0, 1), "add"),
            7: ((1, 0), (1, 1), "add")}
# fold signs: product -> [(quadrant, sign)] ; quadrants 0=C11 1=C12 2=C21
# 3=C22. C11=M1+M4-M5+M7  C12=M3+M5  C21=M2+M4  C22=M1-M2+M3+M6.
# The first product to touch a quadrant (in emission order) is its init.
_FOLD_SIGNS = {1: [(0, +1), (3, +1)], 2: [(2, +1), (3, -1)],
               3: [(1, +1), (3, +1)], 4: [(0, +1), (2, +1)],
               5: [(0, -1), (1, +1)], 6: [(3, +1)], 7: [(0, +1)]}
# order chosen so DVE-heavy products (both combos staged: 1, 6, 7)
# alternate with raw/light ones, and every quadrant's first toucher is +.
_P_ORDER = [1, 3, 6, 4, 7, 2, 5]


def _strassen_blocks(C):
    """Two even blocks, halves in [240, 512] to stay off the LDWEIGHTS
    floor and inside one PSUM bank."""
    assert C % 4 == 0
    b0 = (C // 2 + 2) // 4 * 4
    b1 = C - b0
    assert b0 % 2 == 0 and b1 % 2 == 0
    return [b0, b1]


_FOLDS = {1: [(0, "init"), (3, "init")], 2: [(2, "init"), (3, "sub")],
          3: [(1, "init"), (3, "add")], 4: [(0, "add"), (2, "add")],
          5: [(0, "sub"), (1, "add")], 6: [(3, "add")], 7: [(0, "add")]}


def _build_strassen(C, d=D, h=H):
    """mm1 direct + mm2 with one Strassen level (7/8 of mm2's PE work).

    Why only mm2: a Strassen'd matmul needs its 7 products folded into the
    4 output quadrants by engines reading PSUM, and those folds cost
    ~0.5us per [128,n2] op. mm1 has 16 out-tiles per block (384 fold ops)
    which starves the PE; mm2 has only 4 (96 folds) which fits in the
    vector/scalar engines' idle time.

    Layout: two blocks of ~C/2 tokens; all matmuls run on token HALVES
    (n2~264) since a [128, n] fp32 psum with n>512 exceeds one PSUM bank.
    h lives in one [128, 32, n] tile so mm2's Strassen B-combos (h-half
    sums) batch 4 k-tiles per DVE op. Fold inits are scalar-engine copies;
    the b2 output bias is added on the host.
    """
    key = (C, d, h, "strassen4")
    if key in _program_cache:
        return _program_cache[key]

    nd = d // PART          # 8
    nh = h // PART          # 32
    kh = nh // 2            # 16 k-tiles per h-half (mm2 Strassen)
    mh = 4                  # m-tiles per d-half (mm2 Strassen)
    ns = _strassen_blocks(C)
    xoff = [0, ns[0]]
    N2MAX = (max(ns) // 2 + 7) // 8 * 8

    w1_start = np.concatenate([[0], np.cumsum(W1_CHUNKS)])

    bf16 = mybir.dt.bfloat16
    f32 = mybir.dt.float32

    nc = bacc.Bacc("TRN2", target_bir_lowering=False, debug=False,
                   num_devices=NCORES)

    xt = nc.dram_tensor("xt", [PART, nd * C], bf16, kind="ExternalInput")
    w1ts = [nc.dram_tensor(f"w1t{i}", [PART, nd * ch * PART], bf16,
                           kind="ExternalInput")
            for i, ch in enumerate(W1_CHUNKS)]
    w2qt = [nc.dram_tensor(f"w2q{q}", [PART, kh * 512], bf16,
                           kind="ExternalInput") for q in range(4)]
    b1t = nc.dram_tensor("b1t", [PART, nh], f32, kind="ExternalInput")
    b2t = nc.dram_tensor("b2t", [PART, nd], f32, kind="ExternalInput")
    yt = nc.dram_tensor("yt", [PART, nd * C], bf16, kind="ExternalOutput")

    with tile.TileContext(nc) as tc:
        with (
            tc.tile_pool(name="wpool", bufs=1) as wpool,
            tc.tile_pool(name="xpool", bufs=1) as xpool,
            tc.tile_pool(name="h1pool", bufs=1) as h1pool,
            tc.tile_pool(name="st2pool", bufs=2) as st2pool,
            tc.tile_pool(name="bhpool", bufs=3) as bhpool,
            tc.tile_pool(name="cpool", bufs=1) as cpool,
            tc.tile_pool(name="pspool", bufs=8, space="PSUM") as pspool,
        ):
            warm = wpool.tile([PART, 256], bf16, tag="warm")
            nc.vector.memset(warm[:, :], 0.0)
            wps = pspool.tile([PART, 512], f32, tag="ps", name="wps")
            NWARM = 22  # ends ~12.5us; repacked x j0-set + W1 c0 land ~12.2
            for i in range(NWARM):
                nc.tensor.matmul(wps[:, :256], warm[:, :PART], warm[:, :],
                                 start=(i == 0), stop=(i == NWARM - 1))

            w1_sb = []
            for i, ch in enumerate(W1_CHUNKS):
                t = wpool.tile([PART, nd * ch * PART], bf16, tag=f"w1_{i}")
                w1_sb.append(t)
            w2q = [wpool.tile([PART, kh, 512], bf16, tag=f"w2q{q}",
                              name=f"w2q{q}") for q in range(4)]
            b1_sb = wpool.tile([PART, nh], f32, tag="b1")
            b2_sb = wpool.tile([PART, nd], f32, tag="b2")

            # x packed [j-half][d-tile][half-cols]: the j=0 working set
            # (all the first h-tile chains need) is a contiguous prefix,
            # so it lands ~1.4us earlier than whole-d-tile transfers.
            x_sb = {}
            for b, n in enumerate(ns):
                x_sb[b] = xpool.tile([PART, 2 * nd, n // 2], bf16,
                                     tag="x", name="x")

            # ---- DMA: x0 halves on sync+scalar, weights need-ordered on
            # sync (W1 chunks first, W2 quadrants q0,q3,q2,q1, x block 1).
            n0 = ns[0]
            h0 = (nd // 2) * (n0 // 2)      # cols per (j, d-half) piece
            # sync: j0 d0-3, W1 c0, j1 d0-3, W1 c1.. ; scalar: j0 d4-7,
            # j1 d4-7, biases. First chain gate drops ~13.5us -> ~12.1us.
            nc.sync.dma_start(out=x_sb[0][:, 0:nd // 2, :],
                              in_=xt.ap()[:, 0:h0])
            nc.scalar.dma_start(out=x_sb[0][:, nd // 2:nd, :],
                                in_=xt.ap()[:, h0:2 * h0])
            nc.sync.dma_start(out=w1_sb[0][:, :], in_=w1ts[0].ap()[:, :])
            nc.sync.dma_start(out=x_sb[0][:, nd:nd + nd // 2, :],
                              in_=xt.ap()[:, 2 * h0:3 * h0])
            nc.scalar.dma_start(out=x_sb[0][:, nd + nd // 2:2 * nd, :],
                                in_=xt.ap()[:, 3 * h0:4 * h0])
            nc.scalar.dma_start(out=b1_sb[:, :], in_=b1t.ap()[:, :])
            nc.scalar.dma_start(out=b2_sb[:, :], in_=b2t.ap()[:, :])
            for i in range(1, len(W1_CHUNKS)):
                nc.sync.dma_start(out=w1_sb[i][:, :], in_=w1ts[i].ap()[:, :])
            for q in (0, 3, 2, 1):
                nc.sync.dma_start(out=w2q[q][:, :, :], in_=w2qt[q].ap()[:, :])
            nc.sync.dma_start(out=x_sb[1][:, :, :],
                              in_=xt.ap()[:, nd * ns[0]:nd * C])

            for b, n in enumerate(ns):
                n2 = n // 2
                xb = x_sb[b]
                # one big h tile: [128, h-tile, n]; token halves j in cols
                h1 = h1pool.tile([PART, nh, n], bf16, tag="h1", name="h1")

                # ---------- mm1: direct, per (h-tile, token-half) ----------
                for hi in range(nh):
                    hc = int(np.searchsorted(w1_start, hi, "right")) - 1
                    ho = hi - w1_start[hc]
                    chw = W1_CHUNKS[hc] * PART
                    for j in range(2):
                        ps = pspool.tile([PART, 512], f32, tag="ps",
                                         name="ps")
                        for di in range(nd):
                            nc.tensor.matmul(
                                ps[:, :n2],
                                w1_sb[hc][:, di * chw + ho * PART:
                                          di * chw + (ho + 1) * PART],
                                xb[:, j * nd + di:j * nd + di + 1, :],
                                start=(di == 0), stop=(di == nd - 1),
                            )

                        nc.scalar.activation(
                            h1[:, hi:hi + 1, j * n2:(j + 1) * n2],
                            ps[:, :n2],
                            mybir.ActivationFunctionType.Relu,
                            bias=b1_sb[:, hi:hi + 1], scale=1.0)

                # ---------- mm2: one Strassen level ----------
                def movingh(i, j, kt):
                    return h1[:, i * kh + kt:i * kh + kt + 1,
                              j * n2:(j + 1) * n2]

                yst = [cpool.tile([PART, mh, N2MAX], bf16, tag=f"c{qd}",
                                  name=f"yq{qd}") for qd in range(4)]
                for p in range(1, 8):
                    qa, qb, op = _A_COMBO[p]
                    ps = [pspool.tile([PART, 512], f32, tag="ps", name="ps")
                          for _ in range(mh)]
                    sa, sb_, bop = _B_COMBO[p]
                    for kg in range(0, kh, 4):
                        # batched moving combos for 4 k-tiles at once
                        if sb_ is not None:
                            bh = bhpool.tile([PART, 4, N2MAX], bf16,
                                             tag="bh", name="bh")
                            alu = (mybir.AluOpType.add if bop == "add"
                                   else mybir.AluOpType.subtract)
                            nc.vector.tensor_tensor(
                                bh[:, :, 0:n2],
                                h1[:, sa[0] * kh + kg:sa[0] * kh + kg + 4,
                                   sa[1] * n2:(sa[1] + 1) * n2],
                                h1[:, sb_[0] * kh + kg:sb_[0] * kh + kg + 4,
                                   sb_[1] * n2:(sb_[1] + 1) * n2],
                                alu)
                        # batched stationary combos for 4 k-tiles
                        if qb is not None:
                            st = st2pool.tile([PART, 4, 512], bf16,
                                              tag="st2", name="st2")
                            alu = (mybir.AluOpType.add if op == "add"
                                   else mybir.AluOpType.subtract)
                            nc.vector.tensor_tensor(
                                st[:, :, :], w2q[qa][:, kg:kg + 4, :],
                                w2q[qb][:, kg:kg + 4, :], alu)
                        for ki in range(4):
                            kt = kg + ki
                            if sb_ is None:
                                mv = movingh(sa[0], sa[1], kt)
                            else:
                                mv = bh[:, ki:ki + 1, 0:n2]
                            for m in range(mh):
                                if qb is None:
                                    lhsT = w2q[qa][:, kt:kt + 1,
                                                   m * PART:(m + 1) * PART]
                                else:
                                    lhsT = st[:, ki:ki + 1,
                                              m * PART:(m + 1) * PART]
                                nc.tensor.matmul(
                                    ps[m][:, :n2], lhsT, mv,
                                    start=(kt == 0), stop=(kt == kh - 1))
                    for m in range(mh):
                        for qd, op2 in _FOLDS[p]:
                            dst = yst[qd][:, m:m + 1, 0:n2]
                            if op2 == "init":
                                nc.scalar.copy(dst, ps[m][:, :n2])
                            else:
                                alu2 = (mybir.AluOpType.add if op2 == "add"
                                        else mybir.AluOpType.subtract)
                                nc.vector.tensor_tensor(dst, dst,
                                                        ps[m][:, :n2], alu2)
                for qd in range(4):
                    i, j = qd // 2, qd % 2
                    for m in range(mh):
                        di = i * mh + m
                        ring = nc.sync if b == len(ns) - 1 else nc.gpsimd
                        ring.dma_start(
                            out=yt.ap()[:, nd * xoff[b] + di * n + j * n2:
                                        nd * xoff[b] + di * n
                                        + (j + 1) * n2],
                            in_=yst[qd][:, m:m + 1, 0:n2])

    nc.compile()
    _program_cache[key] = nc
    return nc


def _route(x, bn, Wlg, blg, Wg, k):
    """Numpy replica of the reference routing. Returns per-(g,e) assignment."""
    glog = bn @ Wlg.T + blg                       # (N, G)
    sel_group = np.argmax(glog, axis=1)           # (N,)
    assign = []
    for g in range(Wg.shape[0]):
        logits = x @ Wg[g].T                      # (N, E)
        order = np.argsort(-logits, axis=1, kind="stable")
        sel = order[:, :k]                        # (N, k)
        top = np.take_along_axis(logits, sel, axis=1).astype(np.float32)
        m = top.max(axis=1, keepdims=True)
        ex = np.exp(top - m)
        w = ex / ex.sum(axis=1, keepdims=True)    # (N, k)
        assign.append((sel, w))
    return sel_group, assign


def _pack_x(X, d, ns):
    """(C, d) fp32 -> [128, nd*C] bf16 merged-tile layout, block-major:
    per partition row: [blk][di][token]."""
    nd = d // PART
    xt = X.T.astype(_BF16)                        # (d, C)
    parts = []
    c0 = 0
    for n in ns:
        blk = xt[:, c0:c0 + n].reshape(nd, PART, n).transpose(1, 0, 2)
        parts.append(blk.reshape(PART, nd * n))
        c0 += n
    return np.ascontiguousarray(np.concatenate(parts, axis=1))


def _pack_w1(W1e, d):
    """(h, d) -> per-chunk [128, nd*ch*128] bf16: per partition row
    [di][h cols of chunk]."""
    nd = d // PART
    w = W1e.T.astype(_BF16)                       # (d, h)
    outs = []
    h0 = 0
    for ch in W1_CHUNKS:
        cw = ch * PART
        c = w[:, h0:h0 + cw].reshape(nd, PART, cw).transpose(1, 0, 2)
        outs.append(np.ascontiguousarray(c.reshape(PART, nd * cw)))
        h0 += cw
    return outs


def _pack_w2(W2e, d):
    """(d, h) -> per-chunk [128, ch*d] bf16: per partition row
    [hj][d cols]."""
    w = W2e.T.astype(_BF16)                       # (h, d)
    outs = []
    h0 = 0
    for ch in W2_CHUNKS:
        c = w[h0 * PART:(h0 + ch) * PART, :].reshape(ch, PART, d)
        outs.append(np.ascontiguousarray(
            c.transpose(1, 0, 2).reshape(PART, ch * d)))
        h0 += ch
    return outs


def _pack_x_s(X, d, ns):
    """(C, d) fp32 -> [128, cols] bf16 for the strassen build: per block
    [j-half][d-tile][token-half] so the j=0 working set is a contiguous
    prefix of the block's region."""
    nd_ = d // PART
    xt = X.T.astype(_BF16)                        # (d, C)
    parts = []
    c0 = 0
    for n in ns:
        n2 = n // 2
        b4 = xt[:, c0:c0 + n].reshape(nd_, PART, 2, n2)
        parts.append(np.ascontiguousarray(
            b4.transpose(1, 2, 0, 3).reshape(PART, 2 * nd_ * n2)))
        c0 += n
    return np.ascontiguousarray(np.concatenate(parts, axis=1))


def _pack_w1q(W1e):
    """(H, D) -> 4 quadrant arrays [128, 4mg*4kt*512] bf16 for Strassen.
    dram[q][part, (mg*4+kt)*512+u] = W1[qh*2048+mg*512+u, qd*512+kt*128+part]
    """
    outs = []
    for q in range(4):
        qh, qd = q // 2, q % 2
        sub = W1e[qh * 2048:(qh + 1) * 2048, qd * 512:(qd + 1) * 512]
        arr = sub.reshape(4, 512, 4, PART).transpose(3, 0, 2, 1)
        outs.append(np.ascontiguousarray(
            arr.reshape(PART, 4 * 4 * 512).astype(_BF16)))
    return outs


def _pack_w2q(W2e):
    """(D, H) -> 4 quadrant arrays [128, 16kt*512] bf16 for Strassen.
    dram[q][part, kt*512+u] = W2[qd*512+u, qh*2048+kt*128+part], q=2*qd+qh.
    """
    outs = []
    for q in range(4):
        qd, qh = q // 2, q % 2
        sub = W2e[qd * 512:(qd + 1) * 512, qh * 2048:(qh + 1) * 2048]
        arr = sub.reshape(512, 16, PART).transpose(2, 1, 0)
        outs.append(np.ascontiguousarray(
            arr.reshape(PART, 16 * 512).astype(_BF16)))
    return outs


def _unpack_y(yt, d, ns):
    """[128, nd*C] bf16 -> (d, C) f32."""
    nd = d // PART
    yt = np.asarray(yt)
    if yt.dtype != np.float32:
        yt = yt.astype(np.float32)
    out = np.empty((d, int(sum(ns))), np.float32)
    c0 = 0
    o0 = 0
    for n in ns:
        blk = yt[:, o0:o0 + nd * n].reshape(PART, nd, n).transpose(1, 0, 2)
        out[:, c0:c0 + n] = blk.reshape(d, n)
        c0 += n
        o0 += nd * n
    return out


def kernel(**inputs) -> np.ndarray:
    xs = np.asarray(inputs["xs"], np.float32)
    bn = np.asarray(inputs["bottle_neck"], np.float32)
    Wlg = np.asarray(inputs["Wlg"], np.float32)
    blg = np.asarray(inputs["blg"], np.float32)
    Wg = np.asarray(inputs["Wg"], np.float32)
    W1 = np.asarray(inputs["W1"], np.float32)
    b1 = np.asarray(inputs["b1"], np.float32)
    W2 = np.asarray(inputs["W2"], np.float32)
    b2 = np.asarray(inputs["b2"], np.float32)
    k = int(np.asarray(inputs["top_k"]))

    Bx, Lx, d = xs.shape
    hdim = W1.shape[2]
    N = Bx * Lx
    nh = hdim // PART
    nd = d // PART
    x = xs.reshape(N, d)
    bnf = bn.reshape(N, d)

    sel_group, assign = _route(x, bnf, Wlg, blg, Wg, k)

    # Token sets per (group, expert) core.
    idxs, wgts = [], []
    for c in range(NCORES):
        g, e = divmod(c, E)
        sel, w = assign[g]
        mask = (sel_group == g)[:, None] & (sel == e)
        rows, cols = np.nonzero(mask)
        idxs.append(rows)
        wgts.append(w[rows, cols])

    cnt_max = max(len(i) for i in idxs)
    C = max(PART, -(-cnt_max // 4) * 4)           # pad capacity to 4
    use_strassen = (STRASSEN and d == D and hdim == H
                    and 968 <= C <= 2040)
    if use_strassen:
        ns = _strassen_blocks(C)
        nc = _build_strassen(C, d, hdim)
    else:
        ns = _blocks(C)
        nc = _build(C, d, hdim)

    in_maps = []
    for c in range(NCORES):
        g, e = divmod(c, E)
        cnt = len(idxs[c])
        X = np.zeros((C, d), np.float32)
        if cnt:
            X[:cnt] = x[idxs[c]]
        m = {
            "xt": (_pack_x_s(X, d, ns) if use_strassen
                   else _pack_x(X, d, ns)),
            "b1t": np.ascontiguousarray(b1[g, e].reshape(nh, PART).T),
            "b2t": np.ascontiguousarray(b2[g, e].reshape(nd, PART).T),
        }
        if use_strassen:
            for i, arr in enumerate(_pack_w1(W1[g, e], d)):
                m[f"w1t{i}"] = arr
            for q, arr in enumerate(_pack_w2q(W2[g, e])):
                m[f"w2q{q}"] = arr
        else:
            for i, arr in enumerate(_pack_w1(W1[g, e], d)):
                m[f"w1t{i}"] = arr
            for i, arr in enumerate(_pack_w2(W2[g, e], d)):
                m[f"w2t{i}"] = arr
        in_maps.append(m)

    res = bass_utils.run_bass_kernel_spmd(nc, in_maps, core_ids=list(range(NCORES)))

    out = np.zeros((N, d), np.float32)
    for c in range(NCORES):
        cnt = len(idxs[c])
        if cnt == 0:
            continue
        yc = _unpack_y(res.results[c]["yt"], d, ns)[:, :cnt].T
        if use_strassen:
            # the strassen build leaves the output bias to the host
            g, e = divmod(c, E)
            yc = yc + b2[g, e][None, :]
        out[idxs[c]] += wgts[c][:, None] * yc
    return out.reshape(Bx, Lx, d).astype(np.float32)

